# revision 24
# baseline (speedup 1.0000x reference)
"""Trainium2 Bass kernel for CAM-style channel attention module.

Reference computation (per batch b):
    Q  = W @ X + bias          # 1x1 conv: [256,512]@[512,4096] -> [256,4096]
    E  = Q @ X^T / sqrt(4096)  # [256,512] channel-attention energy
    A  = softmax(E, axis=-1)
    out = gamma * (A @ X) + Q  # residual

Two algebraic tricks:
 1. Residual fusion:  gamma*(A@X) + (W@X + b) = (W + gamma*A) @ X + b
    so the final stage is one fused matmul with combined weights.
 2. Gram factorization of the energy:
        E*64 = Q @ X^T = W @ (X X^T) + b (1^T X^T) = W @ G + b s^T
    G = X X^T is a 512x512 SYMMETRIC matrix: compute only the lower
    triangle on the PE (10/16 of the full cost), mirror the off-diag
    blocks with 6 cheap PE transposes, and recover E with a tiny
    [256,512]@[512,512] matmul.  The row-sum vector s falls out of the
    same matmuls via a ones-column prepended to the host-provided X^T.

Everything runs in fp16 (fp32 PSUM accumulation): fp16 has 8x finer
mantissa than bf16, cutting numeric error ~4x vs a bf16 design, and
the output is written fp16 (half the store traffic).  fp8 was
evaluated and rejected: the softmax here is highly peaked, so logit
noise from e4m3 quantization (~0.07 abs) blows the 2e-2 budget.

Device strategy: 8 NeuronCores, data-parallel over batch, 2 per core.
Host pre-transposes/casts X (both [c,n] fp16 for the final stage and
ones-augmented [n,c] fp16 for the Gram stage) so the device does zero
layout work.  Per-core PE time ~2x34us vs ~2x48us for the direct
Q/E/transpose formulation.

G row passes (lower triangle, xta col 0 = ones -> s segments):
    p0: row3 cols 0:384+s3   p1: row2 0:384+s2   p2: row1 0:256+s1
    p3: row0 0:128+s0        p4: row3 diag 384:512 (no ones)
p4 is last so the s-column is complete one full pass before the E
stage needs s as a [1,512] row (PE transpose + 4 tiny SBUF-to-SBUF
DMA gathers have ~3us of latency to hide).  Mirrors/E-contraction
matmuls are hooked into the middle of later passes' instruction
streams so the PE never waits on an evacuation it just triggered.

PSUM budget (8 banks): psG tag "g" bufs=5 (G passes + E accumulators;
the 7-allocations-per-batch rotation is timed so every bank reuse
lands after the prior group's evacuation), psF tag "o" bufs=3
(warmup / mirrors / A^T / final-stage chunks).
"""

import numpy as np

import concourse.bass as bass
import concourse.tile as tile
from concourse import bacc, mybir
from concourse.bass_utils import run_bass_kernel_spmd

P = 128
NB = 2        # batches per core (B=16 over 8 cores)
C = 512       # input channels
C1 = 256      # conv output channels
HW = 4096     # H*W
CT = C // P   # 4 c-tiles
NT = HW // P  # 32 n-tiles
QT = C1 // P  # 2 q-tiles
XTW = 513     # xta row: [ones | X^T row]
F32 = mybir.dt.float32
F16 = mybir.dt.float16
ESCALE = 1.0 / 64.0  # 1/sqrt(HW)

N_CORES = 8

# (ci, rhs_lo, rhs_hi) over xta columns; xta col 1+c is X^T col c.
G_PASSES = [
    (3, 0, 385),    # s[384:512] + G[3-block, 0:384]
    (2, 0, 385),    # s[256:384] + G[2-block, 0:384]
    (1, 0, 257),    # s[128:256] + G[1-block, 0:256]
    (0, 0, 129),    # s[0:128]   + G[0-block, 0:128]
    (3, 385, 513),  # G[3-block, 384:512] (diag block, no ones col)
]
# hooked items fire mid-tail of pass <key> (0-based), after the
# previous pass's evacuation has had time to complete.
# mirror (dst, src): gsb[:, dst, src-block] <- T(gsb[:, src, dst-block])
G_MIRRORS = {1: [(0, 3), (1, 3), (2, 3)],
             2: [(0, 2), (1, 2)],
             3: [(0, 1)]}
G_ECT = {2: 2, 3: 1, 4: 0}  # pass idx -> E ct emitted mid that pass
# E ct 3 needs p4 (last pass) and is emitted right after emit_G.


def build_nc():
    nc = bacc.Bacc("TRN2", target_bir_lowering=False, debug=False,
                   num_devices=N_CORES)

    xta_d = nc.dram_tensor("xta", [NB, P, NT, XTW], F16,
                           kind="ExternalInput").ap()
    x16_d = nc.dram_tensor("x16", [NB, P, 2, CT, HW // 2], F16,
                           kind="ExternalInput").ap()
    # packed constant buffers: f16 [wt16 | ident | brow(p0)] + f32 [bq|gam]
    pack_d = nc.dram_tensor("pack", [P, 1440], F16,
                            kind="ExternalInput").ap()
    packf_d = nc.dram_tensor("packf", [P, 4], F32,
                             kind="ExternalInput").ap()
    out_d = nc.dram_tensor("out", [NB, C1, HW], F16,
                           kind="ExternalOutput").ap()

    with tile.TileContext(nc) as tc:
        with (
            tc.tile_pool(name="const", bufs=1) as const,
            tc.tile_pool(name="xta_p", bufs=NB) as xta_pool,
            tc.tile_pool(name="x16_p", bufs=NB) as x16_pool,
            tc.tile_pool(name="gsb_p", bufs=NB) as gsb_pool,
            tc.tile_pool(name="sm", bufs=NB) as sm_pool,
            tc.tile_pool(name="svec", bufs=NB) as svec_pool,
            tc.tile_pool(name="lhsf_p", bufs=NB) as lhsf_pool,
            tc.tile_pool(name="osb_p", bufs=3) as osb_pool,
            tc.tile_pool(name="psG", bufs=5, space="PSUM") as psG,
            tc.tile_pool(name="psF", bufs=3, space="PSUM") as psF,
        ):
            out_r = out_d.rearrange("b (t p) n -> b p t n", p=P)

            st = [dict() for _ in range(NB)]

            # ---- input DMAs, all on the sync (SP) queue, in priority
            # order: first xta0 chunks (Gram stage batch 0; small leading
            # chunks so the PE can start at ~7.5us), then the small
            # constants, then the rest of xta0, xta1, and x16 (the final
            # stage consumes x16 much later).
            for bi in range(NB):
                st[bi]["xta"] = xta_pool.tile([P, NT, XTW], F16, tag="xta",
                                              name=f"xta{bi}")
            # Queue plan. Measured DMA behavior: each HWDGE queue
            # serializes its transfers with ~1-1.5us per-DMA latency
            # overhead, sustaining ~350-450GB/s only on big back-to-back
            # transfers; the two queues run in parallel.  So: few, big
            # chunks, split across sync+scalar by consumption deadline.
            #   sync:   xta0[0:4], xta0[8:16], xta0[24:32], xta1[0:16],
            #           x16_0, x16_1
            #   scalar: xta0[4:8], xta0[16:24], consts, xta1[16:32]
            def xta_dma(bi, lo, hi, q):
                q.dma_start(out=st[bi]["xta"][:, lo:hi, :],
                            in_=xta_d[bi][:, lo:hi, :])

            xta_dma(0, 0, 4, nc.sync)
            xta_dma(0, 4, 8, nc.scalar)
            xta_dma(0, 8, 16, nc.sync)
            xta_dma(0, 16, 24, nc.scalar)
            xta_dma(0, 24, 32, nc.scalar)
            xta_dma(1, 0, 16, nc.sync)

            # ---- constants: ONE packed DMA pair (each extra DMA costs
            # ~1.4us of serialized queue time) ----
            pack = const.tile([P, 1440], F16)
            nc.scalar.dma_start(out=pack, in_=pack_d)
            packf = const.tile([P, 4], F32)
            nc.scalar.dma_start(out=packf, in_=packf_d)
            wt16 = pack[:, 0:1024].rearrange("p (ct q) -> p ct q", ct=CT)
            ident = pack[:, 1024:1152]
            brow = pack[0:1, 1156:1412]
            bq = packf[:, 0:2]
            gam = packf[:, 2:3]

            xta_dma(1, 16, 32, nc.scalar)
            for bi in range(NB):
                st[bi]["x16"] = x16_pool.tile([P, 2, CT, HW // 2], F16,
                                              tag="x16", name=f"x16_{bi}")
            for bi in range(NB):
                nc.sync.dma_start(out=st[bi]["x16"], in_=x16_d[bi])

            for bi in range(NB):
                st[bi]["gsb"] = gsb_pool.tile([P, CT, C], F16, tag="gsb",
                                              name=f"gsb{bi}")
                st[bi]["scol"] = svec_pool.tile([P, CT], F16, tag="scol",
                                                name=f"scol{bi}")
                st[bi]["srow"] = svec_pool.tile([1, C], F16, tag="srow",
                                                name=f"srow{bi}")

            # ---- HAM warm-up: dummy matmuls on a memset tile (engine
            # memset beats any DMA producer by ~2us, so the PE starts at
            # ~7us) while the first xta chunks land.  The HAM clock
            # ramps to full only after several us of sustained PE
            # activity, so every us of earlier start and every gap
            # avoided is wall-clock saved.  The warmup psum is never
            # read.
            warm_t = const.tile([P, P], F16)
            nc.vector.memset(warm_t, 1.0)
            ps_w = psF.tile([P, C], F32, tag="o", name="warm")
            NWARM = 48
            for wj in range(NWARM):
                nc.tensor.matmul(ps_w[:, :P], warm_t, warm_t,
                                 start=(wj == 0), stop=(wj == NWARM - 1))

            # ------------- emission helpers ---------------------------
            def ecopy(eng, out, in_):
                if eng is nc.scalar:
                    eng.copy(out=out, in_=in_)
                else:
                    eng.tensor_copy(out=out, in_=in_)

            def g_mm(bi, pi, nt, start, stop):
                ci, lo, hi = G_PASSES[pi]
                xta = st[bi]["xta"]
                nc.tensor.matmul(
                    st[bi]["ps_g"][pi][:, :hi - lo],
                    xta[:, nt, 1 + ci * P:1 + (ci + 1) * P],
                    xta[:, nt, lo:hi],
                    start=start, stop=stop)

            def g_evac(bi, pi, eng):
                ci, lo, hi = G_PASSES[pi]
                ps = st[bi]["ps_g"][pi]
                gsb, scol = st[bi]["gsb"], st[bi]["scol"]
                if lo == 0:
                    ecopy(eng, scol[:, ci:ci + 1], ps[:, 0:1])
                    g0, p0 = 0, 1
                else:
                    g0, p0 = lo - 1, 0
                gw = (hi - lo) - p0
                ecopy(eng, gsb[:, ci, g0:g0 + gw], ps[:, p0:p0 + gw])

            def emit_mirror(bi, dst, src, eng):
                gsb = st[bi]["gsb"]
                ps_m = psF.tile([P, P], F32, tag="o", name=f"ps_m{bi}")
                nc.tensor.matmul(ps_m, gsb[:, src, dst * P:(dst + 1) * P],
                                 ident, start=True, stop=True)
                ecopy(eng, gsb[:, dst, src * P:(src + 1) * P], ps_m)

            def emit_e_ct(bi, ct):
                # E psum group: opened at the first ct, closed by emit_bs
                if "ps_e" not in st[bi]:
                    st[bi]["ps_e"] = [
                        psG.tile([P, C], F32, tag="g", name=f"ps_e{bi}{qi}")
                        for qi in range(QT)]
                    st[bi]["e_started"] = False
                first = not st[bi]["e_started"]
                st[bi]["e_started"] = True
                for qi in range(QT):
                    nc.tensor.matmul(
                        st[bi]["ps_e"][qi],
                        wt16[:, ct, qi * P:(qi + 1) * P],
                        st[bi]["gsb"][:, ct, :],
                        start=first, stop=False)

            def emit_s_chain(bi):
                # scol [128,4] --PE transpose--> [4,128] --evac-->
                # --4 tiny SBUF DMAs (vector queue)--> srow [1,512]
                ps_t = psF.tile([4, P], F32, tag="o", name=f"ps_t{bi}")
                nc.tensor.matmul(ps_t, st[bi]["scol"], ident,
                                 start=True, stop=True)
                stt = svec_pool.tile([4, P], F16, tag="st", name=f"st{bi}")
                nc.vector.tensor_copy(out=stt, in_=ps_t)
                nc.gpsimd.dma_start(out=st[bi]["srow"][0:1, :], in_=stt)

            def emit_bs(bi):
                # rank-1 b s^T accumulated into the E psum; closes group
                for qi in range(QT):
                    nc.tensor.matmul(
                        st[bi]["ps_e"][qi],
                        brow[0:1, qi * P:(qi + 1) * P],
                        st[bi]["srow"][0:1, :],
                        start=False, stop=True)

            def emit_G(bi, split, extra_hooks=None):
                # nt-outer prefix (needs 5 psum banks, DMA-streamable),
                # then per-pass tails with hooked mirror/E interleaves.
                st[bi]["ps_g"] = [
                    psG.tile([P, C], F32, tag="g", name=f"ps_g{bi}{pi}")
                    for pi in range(len(G_PASSES))]
                for nt in range(split):
                    for pi in range(len(G_PASSES)):
                        g_mm(bi, pi, nt, start=(nt == 0), stop=False)
                evac_rr = [nc.vector, nc.vector]
                for pi in range(len(G_PASSES)):
                    tail = list(range(split, NT))
                    for k, nt in enumerate(tail):
                        g_mm(bi, pi, nt, start=(split == 0 and nt == 0),
                             stop=(nt == NT - 1))
                        if k == min(4, len(tail) // 2):
                            for dst, src in G_MIRRORS.get(pi, []):
                                emit_mirror(bi, dst, src, nc.vector)
                            if pi in G_ECT:
                                emit_e_ct(bi, G_ECT[pi])
                            if pi == len(G_PASSES) - 1:
                                emit_s_chain(bi)
                            if extra_hooks and pi in extra_hooks:
                                for fn in extra_hooks[pi]:
                                    fn()
                    g_evac(bi, pi, evac_rr[pi % 2])
                emit_e_ct(bi, 3)

            def emit_softmax(bi):
                a_scaled = sm_pool.tile([P, QT, C], F16, tag="a",
                                        name=f"a_scaled{bi}")
                for qi in range(QT):
                    ps_e = st[bi]["ps_e"][qi]
                    mx = sm_pool.tile([P, 1], F32, tag="mx")
                    nc.vector.reduce_max(mx, ps_e,
                                         axis=mybir.AxisListType.X,
                                         negate=True)
                    nbias = sm_pool.tile([P, 1], F32, tag="nb")
                    nc.vector.tensor_scalar_mul(nbias, mx, ESCALE)
                    a_f = sm_pool.tile([P, C], F32, tag="af")
                    rs = sm_pool.tile([P, 1], F32, tag="rs")
                    nc.scalar.activation(
                        out=a_f, in_=ps_e,
                        func=mybir.ActivationFunctionType.Exp,
                        bias=nbias, scale=ESCALE, accum_out=rs)
                    rc = sm_pool.tile([P, 1], F32, tag="rc")
                    nc.vector.reciprocal(rc, rs)
                    sc = sm_pool.tile([P, 1], F32, tag="sc")
                    nc.vector.tensor_mul(sc, rc, gam)
                    nc.vector.tensor_scalar_mul(a_scaled[:, qi, :], a_f, sc)
                st[bi]["a"] = a_scaled

            def emit_AT_ct(bi, ct):
                if ct == 0:
                    st[bi]["lhsf"] = lhsf_pool.tile([P, CT, C1], F16,
                                                    name=f"lhsf{bi}")
                a_scaled = st[bi]["a"]
                ps_at = psF.tile([P, C1], F32, tag="o", name="ps_at")
                for qi in range(QT):
                    nc.tensor.matmul(
                        ps_at[:, qi * P:(qi + 1) * P],
                        a_scaled[:, qi, ct * P:(ct + 1) * P], ident,
                        start=True, stop=True)
                nc.vector.tensor_add(
                    out=st[bi]["lhsf"][:, ct, :], in0=ps_at,
                    in1=wt16[:, ct, :])

            def emit_AT(bi):
                for ct in range(CT):
                    emit_AT_ct(bi, ct)

            def emit_F(bi, qi, out_q, hooks=None, chunk_dma=False):
                # final = lhsf^T @ X (+b), 8 chunks of 512 per q-tile.
                # chunk_dma=True writes out per 512-chunk instead of per
                # pair -- used for the last group to shorten the tail.
                lhsf = st[bi]["lhsf"]
                x16 = st[bi]["x16"]
                for pair in range(4):
                    o_sb = osb_pool.tile([P, 2 * C], F16, tag="osb")
                    for half in range(2):
                        nch = pair * 2 + half
                        kk, lo = nch // 4, (nch % 4) * C
                        # sub-split the final chunks: smaller evac+DMA
                        # quanta shorten the post-matmul tail
                        fin = chunk_dma and pair == 3
                        nsub = 2 if fin else 1
                        w = C // nsub
                        for sub in range(nsub):
                            ps_o = psF.tile([P, C], F32, tag="o",
                                            name="ps_o")
                            for ct in range(CT):
                                nc.tensor.matmul(
                                    ps_o[:, :w],
                                    lhsf[:, ct, qi * P:(qi + 1) * P],
                                    x16[:, kk, ct, lo + sub * w:
                                        lo + (sub + 1) * w],
                                    start=(ct == 0), stop=(ct == CT - 1))
                            osl = o_sb[:, half * C + sub * w:
                                       half * C + (sub + 1) * w]
                            last = nsub == 2 and half == 1 and sub == 1
                            if last or (half + sub) % 2 == 1:
                                nc.vector.tensor_scalar_add(
                                    osl, ps_o[:, :w], bq[:, qi:qi + 1])
                            else:
                                nc.scalar.add(out=osl, in_=ps_o[:, :w],
                                              add=bq[:, qi:qi + 1])
                            if fin:
                                oq = nc.sync if (last or
                                                 (half + sub) % 2 == 0) \
                                    else nc.scalar
                                oq.dma_start(
                                    out=out_r[bi, :, qi,
                                              nch * C + sub * w:
                                              nch * C + (sub + 1) * w],
                                    in_=osl)
                        if hooks and (pair, half) in hooks:
                            for fn in hooks[(pair, half)]:
                                fn()
                    if chunk_dma and pair < 3:
                        oq = [nc.sync, nc.scalar, nc.sync][pair]
                        oq.dma_start(
                            out=out_r[bi, :, qi,
                                      pair * 2 * C:(pair + 1) * 2 * C],
                            in_=o_sb)
                    elif not chunk_dma:
                        out_q.dma_start(
                            out=out_r[bi, :, qi,
                                      pair * 2 * C:(pair + 1) * 2 * C],
                            in_=o_sb)

            # ------------------- the schedule -------------------------
            # batch 0: nt-outer prefix of 24 (xta0 still streaming in),
            # short pass tails carry the mirror/E interleaves.
            emit_G(0, 24)
            # batch 1: fully pass-outer (xta1 resident by then; 2-3 bank
            # rotation).  bs0+softmax0 hook mid pass-1 of G1 so the s0
            # DMA-gather latency hides under PE work.
            def bs_sm0():
                emit_bs(0)
                emit_softmax(0)
            emit_G(1, 0, extra_hooks={1: [bs_sm0]})
            emit_AT(0)

            def bs_sm1():
                emit_bs(1)
                emit_softmax(1)
            emit_F(0, 0, nc.scalar, hooks={(1, 1): [bs_sm1]})
            emit_F(0, 1, nc.scalar)
            emit_AT(1)
            emit_F(1, 0, nc.sync)
            emit_F(1, 1, nc.sync, chunk_dma=True)
    nc.compile()
    return nc


_NC_CACHE = None


def _get_nc():
    global _NC_CACHE
    if _NC_CACHE is None:
        _NC_CACHE = build_nc()
    return _NC_CACHE


def make_in_maps(x, conv_w, conv_b, gamma):
    B = x.shape[0]
    xs = np.ascontiguousarray(x.reshape(B, C, HW), dtype=np.float32)
    Wm = conv_w.reshape(C1, C).astype(np.float32)
    b_np = conv_b.astype(np.float32)
    pack = np.zeros((P, 1440), dtype=np.float16)
    pack[:, 0:1024] = Wm.T.reshape(CT, P, C1).transpose(1, 0, 2).reshape(
        P, CT * C1)
    pack[:, 1024:1152] = np.eye(P, dtype=np.float16)
    pack[0, 1156:1412] = b_np
    packf = np.zeros((P, 4), dtype=np.float32)
    packf[:, 0:2] = b_np.reshape(QT, P).T
    packf[:, 2] = gamma.astype(np.float32)[0]

    in_maps = []
    for ci in range(N_CORES):
        xta = np.empty((NB, P, NT, XTW), dtype=np.float16)
        x16 = np.empty((NB, P, 2, CT, HW // 2), dtype=np.float16)
        for bi in range(NB):
            Xb = xs[NB * ci + bi]                       # [C, HW] f32
            xta[bi, :, :, 0] = 1.0
            # xta[p, nt, 1+c] = X[c, nt*128+p]
            xta[bi, :, :, 1:] = Xb.reshape(C, NT, P).transpose(2, 1, 0)
            # x16[p, k, ct, m] = X[ct*128+p, k*2048+m]
            x16[bi] = Xb.reshape(CT, P, 2, HW // 2).transpose(1, 2, 0, 3)
        in_maps.append({
            "xta": np.ascontiguousarray(xta),
            "x16": np.ascontiguousarray(x16),
            "pack": pack,
            "packf": packf,
        })
    return in_maps


def kernel(x, conv_w, conv_b, gamma, trace=False):
    """Full inputs in, full output out. Shards batch over 8 NeuronCores."""
    nc = _get_nc()
    in_maps = make_in_maps(x, conv_w, conv_b, gamma)
    res = run_bass_kernel_spmd(nc, in_maps, core_ids=list(range(N_CORES)),
                               trace=trace)
    outs = [np.asarray(r["out"]).astype(np.float32).reshape(NB, C1, 64, 64)
            for r in res.results]
    full = np.concatenate(outs, axis=0)
    if trace:
        kernel.last_results = res
    return full


kernel.last_results = None
